# revision 23
# baseline (speedup 1.0000x reference)
"""EGNN (4-layer) Trainium2 kernel, 8 NeuronCores, edge-sharded.

v2 strategy (vs baseline):
 - Nodes are greedily bin-packed into 160 buckets of 128 (balanced by degree)
   so every (core, window) bucket carries ~equal edge load; T drops and all
   edge-side work shrinks. Host permutes in/out.
 - The edge-MLP first matmul is decomposed: concat([h_r,h_c,rad,ea]) @ W1 =
   P'[row] + Q'[col] + W35.T@feat35, with feat35 = [ea; c_r*c_c] streamed
   from DRAM and W35 = [W1e; -2wr x3] as one 35-row stationary.
 - P'[row] expansion is an accumulating matmul against a host-built one-hot
   indT [node, edge] plane (layer-invariant, streamed from DRAM) - no gather
   and no on-device indicator build.
 - segment_sum uses host-built one-hot ind [edge, node] tiles (streamed).
 - Only Q'[col] needs a dma_gather (SBUF transposed); layer 0's gq and P
   table are computed on the host from the input h, so layer 0 needs no
   AllGather and no gathers; 3 AllGathers total.
 - Node MLP, residual, final LayerNorm on device.
"""

import math
import os
import sys
from contextlib import ExitStack

import numpy as np

sys.path.insert(0, "/opt/trn_rl_repo")

import ml_dtypes  # noqa: E402

BF = ml_dtypes.bfloat16

N = 20000
NCORES = 8
NPCP = 2560         # padded nodes per core
WINS = 20           # node windows of 128 per core
NB = NCORES * WINS  # global buckets
H = 128
DE = 32
L = 4
EPS = 1e-5

_CACHE = {}


def _groups(T):
    out = []
    left = T
    while left > 0:
        g = min(left, 4)
        out.append(g * 128)
        left -= g
    return out


def _build(T, flags):
    import concourse.bacc as bacc
    import concourse.tile as tile
    from concourse import mybir

    eb1_nz, eb2_nz, nb1_nz, nb2_nz, lnb_nz = flags
    EW = T * 128
    EPAD = WINS * EW
    GRPS = _groups(T)

    f32 = mybir.dt.float32
    bf16 = mybir.dt.bfloat16
    i16 = mybir.dt.int16
    AX = mybir.AxisListType.X
    OP = mybir.AluOpType
    AF = mybir.ActivationFunctionType

    nc = bacc.Bacc(
        "TRN2",
        target_bir_lowering=False,
        debug=False,
        enable_asserts=False,
        num_devices=NCORES,
    )

    def din(name, shape, dt):
        return nc.dram_tensor(name, list(shape), dt, kind="ExternalInput").ap()

    hT_d = din("hT", (H, NPCP), bf16)
    scol_d = din("scol", (128, WINS), f32)
    p0_d = din("p0", (128, WINS * H), bf16)
    gq0_d = din("gq0", (128, EPAD), bf16)
    feat_d = din("feat35", (35, EPAD), bf16)
    ind_d = din("indA", (128, EPAD), bf16)
    indT_d = din("indT", (128, EPAD), bf16)
    idxq_d = din("idxq", (128, EPAD // 16), i16)
    w35_d = din("w35", (L, 35, H), bf16)
    w1h_d = din("w1h", (L, H, H), bf16)
    w1c_d = din("w1c", (L, H, H), bf16)
    wrb2_d = din("wrb2", (128, L * H), f32)
    ew2_d = din("ew2", (L, H, H), bf16)
    nw1h_d = din("nw1h", (L, H, H), bf16)
    nw1a_d = din("nw1a", (L, H, H), bf16)
    nw2_d = din("nw2", (L, H, H), bf16)
    eb1_d = din("eb1T", (H, L), f32)
    nb1_d = din("nb1T", (H, L), f32)
    nb2_d = din("nb2T", (H, L), f32)
    eb2b_d = din("eb2b", (L, H, H), f32) if eb2_nz else None
    lng_d = din("lngb", (H, H), f32)
    lnb_d = din("lnbb", (H, H), f32) if lnb_nz else None
    idb_d = din("identb", (H, H), bf16)

    out_d = nc.dram_tensor("out", [NPCP, H], f32, kind="ExternalOutput").ap()

    with ExitStack() as ctx:
        tc = ctx.enter_context(tile.TileContext(nc))
        const = ctx.enter_context(tc.tile_pool(name="const", bufs=1))
        resid = ctx.enter_context(tc.tile_pool(name="resid", bufs=1))
        dram = ctx.enter_context(tc.tile_pool(name="dram", bufs=1, space="DRAM"))
        gpool = ctx.enter_context(tc.tile_pool(name="gpool", bufs=6))
        wpool = ctx.enter_context(tc.tile_pool(name="wpool", bufs=3))
        work = ctx.enter_context(tc.tile_pool(name="work", bufs=4))
        psT = ctx.enter_context(tc.tile_pool(name="psT", bufs=2, space="PSUM"))
        ps512 = ctx.enter_context(tc.tile_pool(name="ps512", bufs=2, space="PSUM"))
        psm2 = ctx.enter_context(tc.tile_pool(name="psm2", bufs=2, space="PSUM"))
        psagg = ctx.enter_context(tc.tile_pool(name="psagg", bufs=2, space="PSUM"))

        sync = nc.sync

        # ---------- constants ----------
        idb_sb = const.tile([H, H], bf16)
        sync.dma_start(out=idb_sb[:], in_=idb_d[:])
        lng_sb = const.tile([H, H], f32)
        sync.dma_start(out=lng_sb[:], in_=lng_d[:])
        lnb_sb = None
        if lnb_nz:
            lnb_sb = const.tile([H, H], f32)
            sync.dma_start(out=lnb_sb[:], in_=lnb_d[:])
        eb1_sb = const.tile([H, L], f32)
        sync.dma_start(out=eb1_sb[:], in_=eb1_d[:])
        nb1_sb = const.tile([H, L], f32)
        sync.dma_start(out=nb1_sb[:], in_=nb1_d[:])
        nb2_sb = const.tile([H, L], f32)
        sync.dma_start(out=nb2_sb[:], in_=nb2_d[:])
        scol_sb = const.tile([128, WINS], f32)
        sync.dma_start(out=scol_sb[:], in_=scol_d[:])
        wrb2_sb = const.tile([128, L * H], f32)
        sync.dma_start(out=wrb2_sb[:], in_=wrb2_d[:])

        def load_w(name, d, p, dt):
            t = const.tile([p, L, H], dt, name=name)
            sync.dma_start(out=t[:], in_=d.rearrange("l k f -> k l f"))
            return t

        w35_sb = load_w("w35_sb", w35_d, 35, bf16)
        w1h_sb = load_w("w1h_sb", w1h_d, H, bf16)
        w1c_sb = load_w("w1c_sb", w1c_d, H, bf16)
        ew2_sb = load_w("ew2_sb", ew2_d, H, bf16)
        nw1h_sb = load_w("nw1h_sb", nw1h_d, H, bf16)
        nw1a_sb = load_w("nw1a_sb", nw1a_d, H, bf16)
        nw2_sb = load_w("nw2_sb", nw2_d, H, bf16)
        eb2b_sb = load_w("eb2b_sb", eb2b_d, H, f32) if eb2_nz else None

        # ---------- resident state ----------
        hT = resid.tile([H, NPCP], bf16)
        sync.dma_start(out=hT[:], in_=hT_d[:])
        zTa = resid.tile([H, NPCP], bf16)
        p_sb = resid.tile([128, WINS * H], bf16)
        idxq_sb = resid.tile([128, EPAD // 16], i16)
        sync.dma_start(out=idxq_sb[:], in_=idxq_d[:])

        # ---------- layers ----------
        qfull_dram = None
        for l in range(L):
            if l == 0:
                sync.dma_start(out=p_sb[:], in_=p0_d[:])


            # edge pass
            for w in range(WINS):
                wsl = slice(w * EW, (w + 1) * EW)
                HW2 = EW // 2
                if l == 0:
                    gqA = gpool.tile([128, 1, HW2], bf16, tag="gqA")
                    sync.dma_start(out=gqA[:, 0, :], in_=gq0_d[:, w * EW : w * EW + HW2])
                    gqB = gpool.tile([128, 1, HW2], bf16, tag="gqB")
                    sync.dma_start(
                        out=gqB[:, 0, :], in_=gq0_d[:, w * EW + HW2 : (w + 1) * EW]
                    )
                else:
                    i0 = w * (EW // 16)
                    gqA = gpool.tile([128, 1, HW2], bf16, tag="gqA")
                    nc.gpsimd.dma_gather(
                        gqA[:], qfull_dram[:], idxq_sb[:, i0 : i0 + EW // 32],
                        HW2, HW2, H, transpose=True, single_packet=False,
                    )
                    gqB = gpool.tile([128, 1, HW2], bf16, tag="gqB")
                    nc.gpsimd.dma_gather(
                        gqB[:], qfull_dram[:],
                        idxq_sb[:, i0 + EW // 32 : i0 + EW // 16],
                        HW2, HW2, H, transpose=True, single_packet=False,
                    )
                find = wpool.tile([35, EW], bf16, tag="find")
                sync.dma_start(out=find[:], in_=feat_d[:, wsl])
                indw = wpool.tile([128, EW], bf16, tag="indw")
                sync.dma_start(out=indw[:], in_=ind_d[:, wsl])
                indT = wpool.tile([128, EW], bf16, tag="indT")
                sync.dma_start(out=indT[:], in_=indT_d[:, wsl])

                pagg = psagg.tile([H, H], f32, tag="pagg")
                pw = p_sb[:, w * H : (w + 1) * H]
                gtile = 0
                off = 0
                for gsz in GRPS:
                    gsl = slice(off, off + gsz)
                    ps1 = ps512.tile([H, 512], f32, tag="big")
                    nc.tensor.matmul(
                        ps1[:, :gsz], w35_sb[:, l, :], find[:, gsl],
                        start=True, stop=False,
                    )
                    nc.tensor.matmul(
                        ps1[:, :gsz], pw, indT[:, gsl],
                        start=False, stop=True,
                    )
                    m1pre = work.tile([H, 512], f32, tag="m1pre")
                    gh = gqA if off < HW2 else gqB
                    go = off if off < HW2 else off - HW2
                    nc.vector.tensor_tensor(
                        m1pre[:, :gsz], ps1[:, :gsz], gh[:, 0, go : go + gsz],
                        OP.add,
                    )
                    m1t = work.tile([H, 512], bf16, tag="m1t")
                    bias1 = eb1_sb[:, l : l + 1] if eb1_nz else 0.0
                    nc.scalar.activation(
                        m1t[:, :gsz], m1pre[:, :gsz], AF.Silu, bias=bias1
                    )
                    pm2 = psm2.tile([H, 512], f32, tag="pm2")
                    for t in range(gsz // 128):
                        tsl = slice(t * 128, (t + 1) * 128)
                        nc.tensor.matmul(
                            pm2[:, t * 128 : (t + 1) * 128], m1t[:, tsl],
                            ew2_sb[:, l, :], start=True, stop=True,
                        )
                    m2s = work.tile([H, 512], bf16, tag="m2s")
                    if eb2_nz:
                        tm2 = work.tile([H, 512], f32, tag="tm2")
                        for t in range(gsz // 128):
                            tsl = slice(t * 128, (t + 1) * 128)
                            nc.vector.tensor_tensor(
                                tm2[:, tsl], pm2[:, tsl], eb2b_sb[:, l, :], OP.add
                            )
                        nc.scalar.activation(m2s[:, :gsz], tm2[:, :gsz], AF.Silu)
                    else:
                        nc.scalar.activation(m2s[:, :gsz], pm2[:, :gsz], AF.Silu)
                    for t in range(gsz // 128):
                        gti = gtile
                        nc.tensor.matmul(
                            pagg[:], m2s[:, t * 128 : (t + 1) * 128],
                            indw[:, gti * 128 : (gti + 1) * 128],
                            start=(gtile == 0), stop=(gtile == T - 1),
                        )
                        gtile += 1
                    off += gsz
                nc.vector.tensor_copy(zTa[:, w * H : (w + 1) * H], pagg[:])

            # node MLP (+ residual), interleaved with next layer's Q tables
            ln = l + 1
            if ln < L:
                qown_dram = dram.tile([NPCP, H], bf16, tag="qown")
            for g5 in range(NPCP // 512):
                sl = slice(g5 * 512, (g5 + 1) * 512)
                psu = ps512.tile([H, 512], f32, tag="big")
                nc.tensor.matmul(
                    psu[:], nw1h_sb[:, l, :], hT[:, sl], start=True, stop=False
                )
                nc.tensor.matmul(
                    psu[:], nw1a_sb[:, l, :], zTa[:, sl], start=False, stop=True
                )
                u = work.tile([H, 512], bf16, tag="u")
                biasn = nb1_sb[:, l : l + 1] if nb1_nz else 0.0
                nc.scalar.activation(u[:], psu[:], AF.Silu, bias=biasn)
                pso = ps512.tile([H, 512], f32, tag="big")
                nc.tensor.matmul(pso[:], nw2_sb[:, l, :], u[:], start=True, stop=True)
                if l == 0:
                    if nb2_nz:
                        nc.vector.tensor_scalar_add(
                            hT[:, sl], pso[:], nb2_sb[:, l : l + 1]
                        )
                    else:
                        nc.vector.tensor_copy(hT[:, sl], pso[:])
                else:
                    if nb2_nz:
                        nc.vector.scalar_tensor_tensor(
                            hT[:, sl], pso[:], nb2_sb[:, l : l + 1], hT[:, sl],
                            OP.add, OP.add,
                        )
                    else:
                        nc.vector.tensor_tensor(hT[:, sl], pso[:], hT[:, sl], OP.add)
                if ln < L:
                    for j in range(4 * g5, 4 * g5 + 4):
                        jsl = slice(j * H, (j + 1) * H)
                        psq = psT.tile([H, H], f32, tag="pt")
                        nc.tensor.matmul(
                            psq[:], hT[:, jsl], w1c_sb[:, ln, :],
                            start=True, stop=True,
                        )
                        qt = work.tile([H, H], bf16, tag="qt")
                        nc.vector.scalar_tensor_tensor(
                            qt[:], wrb2_sb[:, ln * H : (ln + 1) * H],
                            scol_sb[:, j : j + 1], psq[:], OP.mult, OP.add,
                        )
                        sync.dma_start(out=qown_dram[jsl, :], in_=qt[:])
            if ln < L:
                qfull_dram = dram.tile(
                    [NCORES * NPCP, H], bf16, addr_space="Shared", name=f"qf_{ln}"
                )
                nc.gpsimd.collective_compute(
                    "AllGather",
                    mybir.AluOpType.bypass,
                    replica_groups=[list(range(NCORES))],
                    ins=[qown_dram.opt()],
                    outs=[qfull_dram.opt()],
                )
                # P tables for the next layer overlap the AllGather
                for j in range(WINS):
                    jsl = slice(j * H, (j + 1) * H)
                    psp = psT.tile([H, H], f32, tag="pt")
                    nc.tensor.matmul(
                        psp[:], hT[:, jsl], w1h_sb[:, ln, :], start=True, stop=True
                    )
                    nc.vector.scalar_tensor_tensor(
                        p_sb[:, jsl], wrb2_sb[:, ln * H : (ln + 1) * H],
                        scol_sb[:, j : j + 1], psp[:], OP.mult, OP.add,
                    )

        # ---------- LayerNorm + output ----------
        inv = 1.0 / H
        for j in range(WINS):
            jsl = slice(j * H, (j + 1) * H)
            pst = psT.tile([H, H], bf16, tag="pt")
            nc.tensor.transpose(pst[:], hT[:, jsl], idb_sb[:])
            hn = work.tile([H, H], f32, tag="hn")
            nc.vector.tensor_copy(hn[:], pst[:])
            mu = work.tile([H, 1], f32, tag="mu")
            nc.vector.reduce_sum(mu[:], hn[:], AX)
            nc.vector.tensor_scalar_mul(mu[:], mu[:], inv)
            xc = work.tile([H, H], f32, tag="xc")
            nc.vector.tensor_scalar_sub(xc[:], hn[:], mu[:])
            sq = work.tile([H, H], f32, tag="sq")
            nc.vector.tensor_mul(sq[:], xc[:], xc[:])
            var = work.tile([H, 1], f32, tag="var")
            nc.vector.reduce_sum(var[:], sq[:], AX)
            sd = work.tile([H, 1], f32, tag="sd")
            nc.vector.tensor_scalar(sd[:], var[:], inv, EPS, OP.mult, OP.add)
            nc.scalar.activation(sd[:], sd[:], mybir.ActivationFunctionType.Sqrt)
            rstd = work.tile([H, 1], f32, tag="rstd")
            nc.vector.reciprocal(rstd[:], sd[:])
            on = work.tile([H, H], f32, tag="on")
            nc.vector.tensor_scalar_mul(on[:], xc[:], rstd[:])
            nc.vector.tensor_mul(on[:], on[:], lng_sb[:])
            if lnb_nz:
                nc.vector.tensor_add(on[:], on[:], lnb_sb[:])
            sync.dma_start(out=out_d[jsl, :], in_=on[:])

    nc.compile()
    return nc


def _wrap_idx(v):
    """idx i -> [i%16 partition, i//16 free], replicated to 128 partitions."""
    n = v.shape[0]
    t = v.reshape(n // 16, 16).T.astype(np.int16)
    return np.tile(t, (8, 1))


def _balance_nodes(row):
    """Greedy bin-pack nodes into NB buckets of 128 slots, balancing summed
    degree. Returns perm (bucket-slot -> orig node) and inv (orig -> slot)."""
    deg = np.bincount(row, minlength=N)
    order = np.argsort(-deg, kind="stable")
    # round-robin over a heap would be O(N log NB); vectorized approx:
    # deal nodes snake-wise across buckets by descending degree.
    loads = np.zeros(NB, np.int64)
    fill = np.zeros(NB, np.int32)
    assign = np.empty(N, np.int32)
    # process in chunks of NB with alternating direction (snake) - near
    # optimal for balanced sums and O(N) time.
    idx = 0
    d = 1
    border = np.arange(NB)
    while idx < N:
        chunk = order[idx : idx + NB]
        k = len(chunk)
        # order buckets by current load each round
        border = np.argsort(loads + (fill >= 128) * (1 << 40), kind="stable")
        assign[chunk] = border[:k]
        loads[border[:k]] += deg[chunk]
        fill[border[:k]] += 1
        idx += k
        d = -d
    perm = np.full(NB * 128, -1, np.int64)
    inv = np.empty(N, np.int64)
    slot_fill = np.zeros(NB, np.int32)
    for nsm, b in ((order, assign[order]),):
        # vectorized slot assignment
        ordb = np.argsort(b, kind="stable")
        nodes_sorted = nsm[ordb]
        bs = b[ordb]
        starts = np.searchsorted(bs, np.arange(NB))
        pos = np.arange(N) - starts[bs]
        slots = bs * 128 + pos
        perm[slots] = nodes_sorted
        inv[nodes_sorted] = slots
    return perm, inv


def _prepare(inputs):
    """Host-side prep: returns (T, flags, in_maps, perm, valid_node)."""
    h = np.asarray(inputs["h"], np.float32)
    coords = np.asarray(inputs["coords"], np.float32)
    edge_attr = np.asarray(inputs["edge_attr"], np.float32)
    edges = np.asarray(inputs["edges"]).astype(np.int64)
    ew1 = np.asarray(inputs["edge_w1"], np.float32)
    eb1 = np.asarray(inputs["edge_b1"], np.float32)
    ew2 = np.asarray(inputs["edge_w2"], np.float32)
    eb2 = np.asarray(inputs["edge_b2"], np.float32)
    nw1 = np.asarray(inputs["node_w1"], np.float32)
    nb1 = np.asarray(inputs["node_b1"], np.float32)
    nw2 = np.asarray(inputs["node_w2"], np.float32)
    nb2 = np.asarray(inputs["node_b2"], np.float32)
    ln_g = np.asarray(inputs["ln_g"], np.float32)
    ln_b = np.asarray(inputs["ln_b"], np.float32)

    E = edges.shape[1]
    row, col = edges[0], edges[1]

    perm, inv = _balance_nodes(row)
    valid_node = perm >= 0

    # permuted node data, padded to NB*128 = NCORES*NPCP
    hP = np.zeros((NB * 128, H), np.float32)
    hP[valid_node] = h[perm[valid_node]]
    sP = np.zeros(NB * 128, np.float32)
    sP[valid_node] = (coords[perm[valid_node]] ** 2).sum(-1)
    cP = np.zeros((NB * 128, 3), np.float32)
    cP[valid_node] = coords[perm[valid_node]]

    new_r = inv[row]            # bucket-slot of each edge's row
    new_c = inv[col]
    bucket = new_r // 128       # 0..159
    core = bucket // WINS
    wloc = bucket % WINS

    order = np.argsort(bucket, kind="stable")
    counts = np.bincount(bucket, minlength=NB)
    T = int(math.ceil(counts.max() / 128))
    EW = T * 128
    EPAD = WINS * EW

    starts = np.zeros(NB, np.int64)
    starts[1:] = np.cumsum(counts)[:-1]
    pos = np.arange(E) - starts[bucket[order]]
    slot = (wloc[order]) * EW + pos
    ecore = core[order]
    r_s = new_r[order]
    c_s = new_c[order]
    rowrel = (r_s % 128).astype(np.int64)
    tloc = slot // 128          # tile within the core's EPAD space

    # host-built planes
    feat35 = np.zeros((NCORES, 35, EPAD), BF)
    indA = np.zeros((NCORES, 128, EPAD), BF)
    indT = np.zeros((NCORES, 128, EPAD), BF)
    ea_s = edge_attr[order]
    t1_s = cP[r_s] * cP[c_s]
    feat35[ecore, :DE, slot] = ea_s.astype(BF)
    feat35[ecore, DE:, slot] = t1_s.astype(BF)
    indA[ecore, slot % 128, tloc * 128 + rowrel] = np.float32(1.0)
    indT[ecore, rowrel, slot] = np.float32(1.0)

    idxq_v = c_s
    idxq = np.zeros((NCORES, EPAD), np.int64)
    idxq[ecore, slot] = idxq_v

    # weights
    w1h = ew1[:, 0:H, :]
    w1c = ew1[:, H : 2 * H, :]
    wr = ew1[:, 2 * H, :]          # [L, H]
    w1e = ew1[:, 2 * H + 1 :, :]   # [L, DE, H]
    w35 = np.concatenate(
        [w1e, np.repeat((-2.0 * wr)[:, None, :], 3, axis=1)], axis=1
    )                               # [L, 35, H]
    nw1h = nw1[:, :H, :]
    nw1a = nw1[:, H:, :]

    # layer-0 host tables (match device numerics: bf16 h, bf16 weights,
    # f32 accum, + s*wr in f32, cast bf16)
    hbf = hP.astype(BF).astype(np.float32)
    P0 = (hbf @ w1h[0].astype(BF).astype(np.float32)
          + sP[:, None] * wr[0][None, :]).astype(BF)
    Q0 = (hbf @ w1c[0].astype(BF).astype(np.float32)
          + sP[:, None] * wr[0][None, :]).astype(BF)
    gq0_full = Q0[c_s]              # [E, H] bf16

    flags = (
        bool(np.any(eb1)), bool(np.any(eb2)),
        bool(np.any(nb1)), bool(np.any(nb2)), bool(np.any(ln_b)),
    )

    ident = np.eye(H, dtype=np.float32)

    shared = {
        "w35": w35.astype(BF), "w1h": w1h.astype(BF), "w1c": w1c.astype(BF),
        "ew2": ew2.astype(BF),
        "nw1h": nw1h.astype(BF), "nw1a": nw1a.astype(BF), "nw2": nw2.astype(BF),
        "wrb2": np.tile(wr.reshape(1, L * H), (128, 1)).astype(np.float32),
        "eb1T": np.ascontiguousarray(eb1.T), "nb1T": np.ascontiguousarray(nb1.T),
        "nb2T": np.ascontiguousarray(nb2.T),
        "lngb": np.tile(ln_g, (H, 1)).astype(np.float32),
        "identb": ident.astype(BF),
    }
    if flags[1]:
        shared["eb2b"] = np.repeat(eb2[:, None, :], H, axis=1).astype(np.float32)
    if flags[4]:
        shared["lnbb"] = np.tile(ln_b, (H, 1)).astype(np.float32)

    in_maps = []
    for k in range(NCORES):
        ksl = slice(k * NPCP, (k + 1) * NPCP)
        hk = hP[ksl]
        gq0 = np.zeros((128, EPAD), BF)
        sel = ecore == k
        gq0[:, slot[sel]] = gq0_full[sel].T
        m = {
            "hT": np.ascontiguousarray(hk.T).astype(BF),
            "scol": np.ascontiguousarray(
                sP[ksl].reshape(WINS, 128).T
            ).astype(np.float32),
            "p0": np.ascontiguousarray(
                P0[ksl].reshape(WINS, 128, H).transpose(1, 0, 2).reshape(
                    128, WINS * H
                )
            ),
            "gq0": gq0,
            "feat35": feat35[k],
            "indA": indA[k],
            "indT": indT[k],
            "idxq": _wrap_idx(idxq[k]),
        }
        m.update(shared)
        in_maps.append(m)

    return T, flags, in_maps, perm, valid_node


def kernel(**inputs):
    from concourse.bass_utils import run_bass_kernel_spmd

    T, flags, in_maps, perm, valid_node = _prepare(inputs)

    key = (T, flags)
    if key not in _CACHE:
        _CACHE[key] = _build(T, flags)
    nc = _CACHE[key]

    trace = bool(os.environ.get("EGNN_TRACE"))
    kw = {}
    if trace:
        kw = {"trace": True, "tmpdir": os.environ.get("EGNN_TRACE_DIR") or None}
    res = run_bass_kernel_spmd(nc, in_maps, list(range(NCORES)), **kw)
    if trace:
        print(f"HW exec time: {res.exec_time_ns} ns")

    outp = np.concatenate(
        [res.results[k]["out"] for k in range(NCORES)], axis=0
    )
    res_full = np.zeros((N, H), np.float32)
    res_full[perm[valid_node]] = outp[valid_node]
    return res_full


# revision 25
# speedup vs baseline: 1.0112x; 1.0112x over previous
"""EGNN (4-layer) Trainium2 kernel, 8 NeuronCores, edge-sharded.

v2 strategy (vs baseline):
 - Nodes are greedily bin-packed into 160 buckets of 128 (balanced by degree)
   so every (core, window) bucket carries ~equal edge load; T drops and all
   edge-side work shrinks. Host permutes in/out.
 - The edge-MLP first matmul is decomposed: concat([h_r,h_c,rad,ea]) @ W1 =
   P'[row] + Q'[col] + W35.T@feat35, with feat35 = [ea; c_r*c_c] streamed
   from DRAM and W35 = [W1e; -2wr x3] as one 35-row stationary.
 - P'[row] expansion is an accumulating matmul against a host-built one-hot
   indT [node, edge] plane (layer-invariant, streamed from DRAM) - no gather
   and no on-device indicator build.
 - segment_sum uses host-built one-hot ind [edge, node] tiles (streamed).
 - Only Q'[col] needs a dma_gather (SBUF transposed); layer 0's gq and P
   table are computed on the host from the input h, so layer 0 needs no
   AllGather and no gathers; 3 AllGathers total.
 - Node MLP, residual, final LayerNorm on device.
"""

import math
import os
import sys
from contextlib import ExitStack

import numpy as np

sys.path.insert(0, "/opt/trn_rl_repo")

import ml_dtypes  # noqa: E402

BF = ml_dtypes.bfloat16

N = 20000
NCORES = 8
NPCP = 2560         # padded nodes per core
WINS = 20           # node windows of 128 per core
NB = NCORES * WINS  # global buckets
H = 128
DE = 32
L = 4
EPS = 1e-5

_CACHE = {}


def _groups(T):
    out = []
    left = T
    while left > 0:
        g = min(left, 4)
        out.append(g * 128)
        left -= g
    return out


def _build(T, flags):
    import concourse.bacc as bacc
    import concourse.tile as tile
    from concourse import mybir

    eb1_nz, eb2_nz, nb1_nz, nb2_nz, lnb_nz = flags
    EW = T * 128
    EPAD = WINS * EW
    GRPS = _groups(T)

    f32 = mybir.dt.float32
    bf16 = mybir.dt.bfloat16
    i16 = mybir.dt.int16
    AX = mybir.AxisListType.X
    OP = mybir.AluOpType
    AF = mybir.ActivationFunctionType

    nc = bacc.Bacc(
        "TRN2",
        target_bir_lowering=False,
        debug=False,
        enable_asserts=False,
        num_devices=NCORES,
    )

    def din(name, shape, dt):
        return nc.dram_tensor(name, list(shape), dt, kind="ExternalInput").ap()

    hT_d = din("hT", (H, NPCP), bf16)
    scol_d = din("scol", (128, WINS), f32)
    p0_d = din("p0", (128, WINS * H), bf16)
    gq0_d = din("gq0", (128, EPAD), bf16)
    feat_d = din("feat35", (35, EPAD), bf16)
    ind_d = din("indA", (128, EPAD), bf16)
    indT_d = din("indT", (128, EPAD), bf16)
    idxq_d = din("idxq", (128, EPAD // 16), i16)
    w35_d = din("w35", (L, 35, H), bf16)
    w1h_d = din("w1h", (L, H, H), bf16)
    w1c_d = din("w1c", (L, H, H), bf16)
    wrb2_d = din("wrb2", (128, L * H), f32)
    ew2_d = din("ew2", (L, H, H), bf16)
    nw1h_d = din("nw1h", (L, H, H), bf16)
    nw1a_d = din("nw1a", (L, H, H), bf16)
    nw2_d = din("nw2", (L, H, H), bf16)
    eb1_d = din("eb1T", (H, L), f32)
    nb1_d = din("nb1T", (H, L), f32)
    nb2_d = din("nb2T", (H, L), f32)
    eb2b_d = din("eb2b", (L, H, H), f32) if eb2_nz else None
    lng_d = din("lngb", (H, H), f32)
    lnb_d = din("lnbb", (H, H), f32) if lnb_nz else None
    idb_d = din("identb", (H, H), bf16)

    out_d = nc.dram_tensor("out", [NPCP, H], f32, kind="ExternalOutput").ap()

    with ExitStack() as ctx:
        tc = ctx.enter_context(tile.TileContext(nc))
        const = ctx.enter_context(tc.tile_pool(name="const", bufs=1))
        resid = ctx.enter_context(tc.tile_pool(name="resid", bufs=1))
        dram = ctx.enter_context(tc.tile_pool(name="dram", bufs=1, space="DRAM"))
        gpool = ctx.enter_context(tc.tile_pool(name="gpool", bufs=6))
        wpool = ctx.enter_context(tc.tile_pool(name="wpool", bufs=3))
        work = ctx.enter_context(tc.tile_pool(name="work", bufs=4))
        psT = ctx.enter_context(tc.tile_pool(name="psT", bufs=1, space="PSUM"))
        ps512 = ctx.enter_context(tc.tile_pool(name="ps512", bufs=3, space="PSUM"))
        psm2 = ctx.enter_context(tc.tile_pool(name="psm2", bufs=2, space="PSUM"))
        psagg = ctx.enter_context(tc.tile_pool(name="psagg", bufs=2, space="PSUM"))

        sync = nc.sync

        # ---------- constants ----------
        idb_sb = const.tile([H, H], bf16)
        sync.dma_start(out=idb_sb[:], in_=idb_d[:])
        lng_sb = const.tile([H, H], f32)
        sync.dma_start(out=lng_sb[:], in_=lng_d[:])
        lnb_sb = None
        if lnb_nz:
            lnb_sb = const.tile([H, H], f32)
            sync.dma_start(out=lnb_sb[:], in_=lnb_d[:])
        eb1_sb = const.tile([H, L], f32)
        sync.dma_start(out=eb1_sb[:], in_=eb1_d[:])
        nb1_sb = const.tile([H, L], f32)
        sync.dma_start(out=nb1_sb[:], in_=nb1_d[:])
        nb2_sb = const.tile([H, L], f32)
        sync.dma_start(out=nb2_sb[:], in_=nb2_d[:])
        scol_sb = const.tile([128, WINS], f32)
        sync.dma_start(out=scol_sb[:], in_=scol_d[:])
        wrb2_sb = const.tile([128, L * H], f32)
        sync.dma_start(out=wrb2_sb[:], in_=wrb2_d[:])

        def load_w(name, d, p, dt):
            t = const.tile([p, L, H], dt, name=name)
            sync.dma_start(out=t[:], in_=d.rearrange("l k f -> k l f"))
            return t

        w35_sb = load_w("w35_sb", w35_d, 35, bf16)
        w1h_sb = load_w("w1h_sb", w1h_d, H, bf16)
        w1c_sb = load_w("w1c_sb", w1c_d, H, bf16)
        ew2_sb = load_w("ew2_sb", ew2_d, H, bf16)
        nw1h_sb = load_w("nw1h_sb", nw1h_d, H, bf16)
        nw1a_sb = load_w("nw1a_sb", nw1a_d, H, bf16)
        nw2_sb = load_w("nw2_sb", nw2_d, H, bf16)
        eb2b_sb = load_w("eb2b_sb", eb2b_d, H, f32) if eb2_nz else None

        # ---------- resident state ----------
        hT = resid.tile([H, NPCP], bf16)
        sync.dma_start(out=hT[:], in_=hT_d[:])
        zTa = resid.tile([H, NPCP], bf16)
        p_sb = resid.tile([128, WINS * H], bf16)
        idxq_sb = resid.tile([128, EPAD // 16], i16)
        sync.dma_start(out=idxq_sb[:], in_=idxq_d[:])

        # ---------- layers ----------
        for l in range(L):
            if l == 0:
                sync.dma_start(out=p_sb[:], in_=p0_d[:])
            else:
                # Q tables first so the AllGather can start ASAP
                qown_dram = dram.tile([NPCP, H], bf16, tag="qown")
                for j in range(WINS):
                    jsl = slice(j * H, (j + 1) * H)
                    psq = psT.tile([H, H], f32, tag="pt")
                    nc.tensor.matmul(
                        psq[:], hT[:, jsl], w1c_sb[:, l, :], start=True, stop=True
                    )
                    qt = work.tile([H, H], bf16, tag="qt")
                    nc.vector.scalar_tensor_tensor(
                        qt[:], wrb2_sb[:, l * H : (l + 1) * H],
                        scol_sb[:, j : j + 1], psq[:], OP.mult, OP.add,
                    )
                    sync.dma_start(out=qown_dram[jsl, :], in_=qt[:])

                qfull_dram = dram.tile(
                    [NCORES * NPCP, H], bf16, addr_space="Shared", name=f"qf_{l}"
                )
                nc.gpsimd.collective_compute(
                    "AllGather",
                    mybir.AluOpType.bypass,
                    replica_groups=[list(range(NCORES))],
                    ins=[qown_dram.opt()],
                    outs=[qfull_dram.opt()],
                )
                # P tables overlap the AllGather
                for j in range(WINS):
                    jsl = slice(j * H, (j + 1) * H)
                    psp = psT.tile([H, H], f32, tag="pt")
                    nc.tensor.matmul(
                        psp[:], hT[:, jsl], w1h_sb[:, l, :], start=True, stop=True
                    )
                    nc.vector.scalar_tensor_tensor(
                        p_sb[:, jsl], wrb2_sb[:, l * H : (l + 1) * H],
                        scol_sb[:, j : j + 1], psp[:], OP.mult, OP.add,
                    )


            # edge pass
            for w in range(WINS):
                wsl = slice(w * EW, (w + 1) * EW)
                if l == 0:
                    gq = gpool.tile([128, 1, EW], bf16, tag="gq")
                    sync.dma_start(out=gq[:, 0, :], in_=gq0_d[:, wsl])
                else:
                    isl = slice(w * (EW // 16), (w + 1) * (EW // 16))
                    gq = gpool.tile([128, 1, EW], bf16, tag="gq")
                    nc.gpsimd.dma_gather(
                        gq[:],
                        qfull_dram[:],
                        idxq_sb[:, isl],
                        EW,
                        EW,
                        H,
                        transpose=True,
                        single_packet=False,
                    )
                find = wpool.tile([35, EW], bf16, tag="find")
                sync.dma_start(out=find[:], in_=feat_d[:, wsl])
                indw = wpool.tile([128, EW], bf16, tag="indw")
                sync.dma_start(out=indw[:], in_=ind_d[:, wsl])
                indT = wpool.tile([128, EW], bf16, tag="indT")
                sync.dma_start(out=indT[:], in_=indT_d[:, wsl])

                pagg = psagg.tile([H, H], f32, tag="pagg")
                pw = p_sb[:, w * H : (w + 1) * H]
                gtile = 0
                off = 0
                for gsz in GRPS:
                    gsl = slice(off, off + gsz)
                    ps1 = ps512.tile([H, 512], f32, tag="big")
                    nc.tensor.matmul(
                        ps1[:, :gsz], w35_sb[:, l, :], find[:, gsl],
                        start=True, stop=False,
                    )
                    nc.tensor.matmul(
                        ps1[:, :gsz], pw, indT[:, gsl],
                        start=False, stop=True,
                    )
                    m1pre = work.tile([H, 512], f32, tag="m1pre")
                    nc.vector.tensor_tensor(
                        m1pre[:, :gsz], ps1[:, :gsz], gq[:, 0, gsl], OP.add
                    )
                    m1t = work.tile([H, 512], bf16, tag="m1t")
                    bias1 = eb1_sb[:, l : l + 1] if eb1_nz else 0.0
                    nc.scalar.activation(
                        m1t[:, :gsz], m1pre[:, :gsz], AF.Silu, bias=bias1
                    )
                    pm2 = psm2.tile([H, 512], f32, tag="pm2")
                    for t in range(gsz // 128):
                        tsl = slice(t * 128, (t + 1) * 128)
                        nc.tensor.matmul(
                            pm2[:, t * 128 : (t + 1) * 128], m1t[:, tsl],
                            ew2_sb[:, l, :], start=True, stop=True,
                        )
                    m2s = work.tile([H, 512], bf16, tag="m2s")
                    if eb2_nz:
                        tm2 = work.tile([H, 512], f32, tag="tm2")
                        for t in range(gsz // 128):
                            tsl = slice(t * 128, (t + 1) * 128)
                            nc.vector.tensor_tensor(
                                tm2[:, tsl], pm2[:, tsl], eb2b_sb[:, l, :], OP.add
                            )
                        nc.scalar.activation(m2s[:, :gsz], tm2[:, :gsz], AF.Silu)
                    else:
                        nc.scalar.activation(m2s[:, :gsz], pm2[:, :gsz], AF.Silu)
                    for t in range(gsz // 128):
                        gti = gtile
                        nc.tensor.matmul(
                            pagg[:], m2s[:, t * 128 : (t + 1) * 128],
                            indw[:, gti * 128 : (gti + 1) * 128],
                            start=(gtile == 0), stop=(gtile == T - 1),
                        )
                        gtile += 1
                    off += gsz
                nc.vector.tensor_copy(zTa[:, w * H : (w + 1) * H], pagg[:])

            # node MLP (+ residual)
            for g5 in range(NPCP // 512):
                sl = slice(g5 * 512, (g5 + 1) * 512)
                psu = ps512.tile([H, 512], f32, tag="big")
                nc.tensor.matmul(
                    psu[:], nw1h_sb[:, l, :], hT[:, sl], start=True, stop=False
                )
                nc.tensor.matmul(
                    psu[:], nw1a_sb[:, l, :], zTa[:, sl], start=False, stop=True
                )
                u = work.tile([H, 512], bf16, tag="u")
                biasn = nb1_sb[:, l : l + 1] if nb1_nz else 0.0
                nc.scalar.activation(u[:], psu[:], AF.Silu, bias=biasn)
                pso = ps512.tile([H, 512], f32, tag="big")
                nc.tensor.matmul(pso[:], nw2_sb[:, l, :], u[:], start=True, stop=True)
                if l == 0:
                    if nb2_nz:
                        nc.vector.tensor_scalar_add(
                            hT[:, sl], pso[:], nb2_sb[:, l : l + 1]
                        )
                    else:
                        nc.vector.tensor_copy(hT[:, sl], pso[:])
                else:
                    if nb2_nz:
                        nc.vector.scalar_tensor_tensor(
                            hT[:, sl], pso[:], nb2_sb[:, l : l + 1], hT[:, sl],
                            OP.add, OP.add,
                        )
                    else:
                        nc.vector.tensor_tensor(hT[:, sl], pso[:], hT[:, sl], OP.add)

        # ---------- LayerNorm + output ----------
        inv = 1.0 / H
        for j in range(WINS):
            jsl = slice(j * H, (j + 1) * H)
            pst = psT.tile([H, H], bf16, tag="pt")
            nc.tensor.transpose(pst[:], hT[:, jsl], idb_sb[:])
            hn = work.tile([H, H], f32, tag="hn")
            nc.vector.tensor_copy(hn[:], pst[:])
            mu = work.tile([H, 1], f32, tag="mu")
            nc.vector.reduce_sum(mu[:], hn[:], AX)
            nc.vector.tensor_scalar_mul(mu[:], mu[:], inv)
            xc = work.tile([H, H], f32, tag="xc")
            nc.vector.tensor_scalar_sub(xc[:], hn[:], mu[:])
            sq = work.tile([H, H], f32, tag="sq")
            nc.vector.tensor_mul(sq[:], xc[:], xc[:])
            var = work.tile([H, 1], f32, tag="var")
            nc.vector.reduce_sum(var[:], sq[:], AX)
            sd = work.tile([H, 1], f32, tag="sd")
            nc.vector.tensor_scalar(sd[:], var[:], inv, EPS, OP.mult, OP.add)
            nc.scalar.activation(sd[:], sd[:], mybir.ActivationFunctionType.Sqrt)
            rstd = work.tile([H, 1], f32, tag="rstd")
            nc.vector.reciprocal(rstd[:], sd[:])
            on = work.tile([H, H], f32, tag="on")
            nc.vector.tensor_scalar_mul(on[:], xc[:], rstd[:])
            nc.vector.tensor_mul(on[:], on[:], lng_sb[:])
            if lnb_nz:
                nc.vector.tensor_add(on[:], on[:], lnb_sb[:])
            sync.dma_start(out=out_d[jsl, :], in_=on[:])

    nc.compile()
    return nc


def _wrap_idx(v):
    """idx i -> [i%16 partition, i//16 free], replicated to 128 partitions."""
    n = v.shape[0]
    t = v.reshape(n // 16, 16).T.astype(np.int16)
    return np.tile(t, (8, 1))


def _balance_nodes(row):
    """Greedy bin-pack nodes into NB buckets of 128 slots, balancing summed
    degree. Returns perm (bucket-slot -> orig node) and inv (orig -> slot)."""
    deg = np.bincount(row, minlength=N)
    order = np.argsort(-deg, kind="stable")
    # round-robin over a heap would be O(N log NB); vectorized approx:
    # deal nodes snake-wise across buckets by descending degree.
    loads = np.zeros(NB, np.int64)
    fill = np.zeros(NB, np.int32)
    assign = np.empty(N, np.int32)
    # process in chunks of NB with alternating direction (snake) - near
    # optimal for balanced sums and O(N) time.
    idx = 0
    d = 1
    border = np.arange(NB)
    while idx < N:
        chunk = order[idx : idx + NB]
        k = len(chunk)
        # order buckets by current load each round
        border = np.argsort(loads + (fill >= 128) * (1 << 40), kind="stable")
        assign[chunk] = border[:k]
        loads[border[:k]] += deg[chunk]
        fill[border[:k]] += 1
        idx += k
        d = -d
    perm = np.full(NB * 128, -1, np.int64)
    inv = np.empty(N, np.int64)
    slot_fill = np.zeros(NB, np.int32)
    for nsm, b in ((order, assign[order]),):
        # vectorized slot assignment
        ordb = np.argsort(b, kind="stable")
        nodes_sorted = nsm[ordb]
        bs = b[ordb]
        starts = np.searchsorted(bs, np.arange(NB))
        pos = np.arange(N) - starts[bs]
        slots = bs * 128 + pos
        perm[slots] = nodes_sorted
        inv[nodes_sorted] = slots
    return perm, inv


def _prepare(inputs):
    """Host-side prep: returns (T, flags, in_maps, perm, valid_node)."""
    h = np.asarray(inputs["h"], np.float32)
    coords = np.asarray(inputs["coords"], np.float32)
    edge_attr = np.asarray(inputs["edge_attr"], np.float32)
    edges = np.asarray(inputs["edges"]).astype(np.int64)
    ew1 = np.asarray(inputs["edge_w1"], np.float32)
    eb1 = np.asarray(inputs["edge_b1"], np.float32)
    ew2 = np.asarray(inputs["edge_w2"], np.float32)
    eb2 = np.asarray(inputs["edge_b2"], np.float32)
    nw1 = np.asarray(inputs["node_w1"], np.float32)
    nb1 = np.asarray(inputs["node_b1"], np.float32)
    nw2 = np.asarray(inputs["node_w2"], np.float32)
    nb2 = np.asarray(inputs["node_b2"], np.float32)
    ln_g = np.asarray(inputs["ln_g"], np.float32)
    ln_b = np.asarray(inputs["ln_b"], np.float32)

    E = edges.shape[1]
    row, col = edges[0], edges[1]

    perm, inv = _balance_nodes(row)
    valid_node = perm >= 0

    # permuted node data, padded to NB*128 = NCORES*NPCP
    hP = np.zeros((NB * 128, H), np.float32)
    hP[valid_node] = h[perm[valid_node]]
    sP = np.zeros(NB * 128, np.float32)
    sP[valid_node] = (coords[perm[valid_node]] ** 2).sum(-1)
    cP = np.zeros((NB * 128, 3), np.float32)
    cP[valid_node] = coords[perm[valid_node]]

    new_r = inv[row]            # bucket-slot of each edge's row
    new_c = inv[col]
    bucket = new_r // 128       # 0..159
    core = bucket // WINS
    wloc = bucket % WINS

    order = np.argsort(bucket, kind="stable")
    counts = np.bincount(bucket, minlength=NB)
    T = int(math.ceil(counts.max() / 128))
    EW = T * 128
    EPAD = WINS * EW

    starts = np.zeros(NB, np.int64)
    starts[1:] = np.cumsum(counts)[:-1]
    pos = np.arange(E) - starts[bucket[order]]
    slot = (wloc[order]) * EW + pos
    ecore = core[order]
    r_s = new_r[order]
    c_s = new_c[order]
    rowrel = (r_s % 128).astype(np.int64)
    tloc = slot // 128          # tile within the core's EPAD space

    # host-built planes
    feat35 = np.zeros((NCORES, 35, EPAD), BF)
    indA = np.zeros((NCORES, 128, EPAD), BF)
    indT = np.zeros((NCORES, 128, EPAD), BF)
    ea_s = edge_attr[order]
    t1_s = cP[r_s] * cP[c_s]
    feat35[ecore, :DE, slot] = ea_s.astype(BF)
    feat35[ecore, DE:, slot] = t1_s.astype(BF)
    indA[ecore, slot % 128, tloc * 128 + rowrel] = np.float32(1.0)
    indT[ecore, rowrel, slot] = np.float32(1.0)

    idxq_v = c_s
    idxq = np.zeros((NCORES, EPAD), np.int64)
    idxq[ecore, slot] = idxq_v

    # weights
    w1h = ew1[:, 0:H, :]
    w1c = ew1[:, H : 2 * H, :]
    wr = ew1[:, 2 * H, :]          # [L, H]
    w1e = ew1[:, 2 * H + 1 :, :]   # [L, DE, H]
    w35 = np.concatenate(
        [w1e, np.repeat((-2.0 * wr)[:, None, :], 3, axis=1)], axis=1
    )                               # [L, 35, H]
    nw1h = nw1[:, :H, :]
    nw1a = nw1[:, H:, :]

    # layer-0 host tables (match device numerics: bf16 h, bf16 weights,
    # f32 accum, + s*wr in f32, cast bf16)
    hbf = hP.astype(BF).astype(np.float32)
    P0 = (hbf @ w1h[0].astype(BF).astype(np.float32)
          + sP[:, None] * wr[0][None, :]).astype(BF)
    Q0 = (hbf @ w1c[0].astype(BF).astype(np.float32)
          + sP[:, None] * wr[0][None, :]).astype(BF)
    gq0_full = Q0[c_s]              # [E, H] bf16

    flags = (
        bool(np.any(eb1)), bool(np.any(eb2)),
        bool(np.any(nb1)), bool(np.any(nb2)), bool(np.any(ln_b)),
    )

    ident = np.eye(H, dtype=np.float32)

    shared = {
        "w35": w35.astype(BF), "w1h": w1h.astype(BF), "w1c": w1c.astype(BF),
        "ew2": ew2.astype(BF),
        "nw1h": nw1h.astype(BF), "nw1a": nw1a.astype(BF), "nw2": nw2.astype(BF),
        "wrb2": np.tile(wr.reshape(1, L * H), (128, 1)).astype(np.float32),
        "eb1T": np.ascontiguousarray(eb1.T), "nb1T": np.ascontiguousarray(nb1.T),
        "nb2T": np.ascontiguousarray(nb2.T),
        "lngb": np.tile(ln_g, (H, 1)).astype(np.float32),
        "identb": ident.astype(BF),
    }
    if flags[1]:
        shared["eb2b"] = np.repeat(eb2[:, None, :], H, axis=1).astype(np.float32)
    if flags[4]:
        shared["lnbb"] = np.tile(ln_b, (H, 1)).astype(np.float32)

    in_maps = []
    for k in range(NCORES):
        ksl = slice(k * NPCP, (k + 1) * NPCP)
        hk = hP[ksl]
        gq0 = np.zeros((128, EPAD), BF)
        sel = ecore == k
        gq0[:, slot[sel]] = gq0_full[sel].T
        m = {
            "hT": np.ascontiguousarray(hk.T).astype(BF),
            "scol": np.ascontiguousarray(
                sP[ksl].reshape(WINS, 128).T
            ).astype(np.float32),
            "p0": np.ascontiguousarray(
                P0[ksl].reshape(WINS, 128, H).transpose(1, 0, 2).reshape(
                    128, WINS * H
                )
            ),
            "gq0": gq0,
            "feat35": feat35[k],
            "indA": indA[k],
            "indT": indT[k],
            "idxq": _wrap_idx(idxq[k]),
        }
        m.update(shared)
        in_maps.append(m)

    return T, flags, in_maps, perm, valid_node


def kernel(**inputs):
    from concourse.bass_utils import run_bass_kernel_spmd

    T, flags, in_maps, perm, valid_node = _prepare(inputs)

    key = (T, flags)
    if key not in _CACHE:
        _CACHE[key] = _build(T, flags)
    nc = _CACHE[key]

    trace = bool(os.environ.get("EGNN_TRACE"))
    kw = {}
    if trace:
        kw = {"trace": True, "tmpdir": os.environ.get("EGNN_TRACE_DIR") or None}
    res = run_bass_kernel_spmd(nc, in_maps, list(range(NCORES)), **kw)
    if trace:
        print(f"HW exec time: {res.exec_time_ns} ns")

    outp = np.concatenate(
        [res.results[k]["out"] for k in range(NCORES)], axis=0
    )
    res_full = np.zeros((N, H), np.float32)
    res_full[perm[valid_node]] = outp[valid_node]
    return res_full
